# revision 19
# baseline (speedup 1.0000x reference)
"""Trainium2 Bass kernel for nn_Attention_60885456388891 (gnn_message_passing).

Computation (per batch b):
  node_h = h @ W_h2node + b_h2node
  score_n[n] = sum_d tanh(p_node_feats[b,n,d] + node_h[b,d]) * w_alpha1[d]
  node_w = renorm(softmax(score_n) * att_masks)
  node_res_ = sum_n node_w[n] * node_feats[b,n,:]
  (same for relations)
  node_res = glu(cat(node_res_, rela_res_) @ W_ng + b_ng)
  rela_res = glu(cat(rela_res_, node_res) @ W_rg + b_rg)

Strategy: pure data-parallel over batch B=512 across 8 cores (64 batches/core).
Memory-bound: all four big streams (pnf/nf/prf/rf) are cast to bf16 on the
host (free) and streamed once -> 48 MiB/core at DMA line rate (~158us floor).

Per-core pipeline (bf16):
  - pnf/prf arrive D-MAJOR ([128(d-chunk), PAIR, KC, N]) so the node_h add is
    a free-dim broadcast on DVE at 2x rate (node_h kept duplicated-pair bf16)
  - node+rela tanh args merged into one [128, PAIR, KC, 384] tile; ONE tanh
    ACT op per PAIR block
  - score matmuls: one-hot w_alpha1|w_alpha2 columns [128, 16] accumulate
    scores batch-major into [16, 384] PSUM (rows 0:8 node, 8:16 rela) --
    one LDW+MM per (batch, chunk), no reduce ops, no fwd transposes
  - ONE fused masked softmax over [16, 384] (host-fused zero-padded mask);
    weights transposed back and DIAGONALIZED into [128, G, G] so phase C's
    weighted sums accumulate all G batch rows into one [G, D] PSUM bank ->
    a single copy per group
  - phase A(g) / phase C(g-1) interleaved at PAIR-block level to keep PE fed
  - early-needed DMAs (projection weights) lead the sync queue; GLU weights,
    one-hots and masks ride the idle gpsimd queue
"""

import numpy as np
import ml_dtypes

import concourse.bass as bass
import concourse.bacc as bacc
import concourse.mybir as mybir
import concourse.tile as tile
from concourse.bass_utils import run_bass_kernel_spmd

# Problem dims (hardcoded per contract)
B, N, R, D = 512, 128, 256, 512
NR = N + R                # 384 fused score columns
NCORES = 8
BS = B // NCORES          # 64 batches per core
GROUPS = 8                # softmax groups per core
G = BS // GROUPS          # 8 batches per group
G2 = 2 * G                # 16 = node rows + rela rows in fused score tile
PAIR = 4                  # batches per stream DMA block
NBLK = BS // PAIR         # 16 stream blocks
KC = D // 128             # 4 d-chunks of 128
KC2 = 2 * D // 128        # 8 k-chunks for the 1024-wide GLU matmuls

F32 = mybir.dt.float32
BF16 = mybir.dt.bfloat16
I32 = mybir.dt.int32
AF = mybir.ActivationFunctionType
ALU = mybir.AluOpType
AX = mybir.AxisListType

NPBF = ml_dtypes.bfloat16


def _ap(t):
    """Tile or AP -> AP covering the whole tile."""
    if isinstance(t, bass.AP):
        return t
    return t[:]


def _view(t, off_elems, dims):
    """Reshape a tile's free dims: keep partition dim, replace free AP.

    dims: list of [step, num] pairs (innermost last), offset in elements
    added to the tile's base offset.
    """
    a = _ap(t)
    return bass.AP(tensor=a.tensor, offset=a.offset + off_elems,
                   ap=[a.ap[0]] + dims)


def build_program():
    nc = bacc.Bacc("TRN2", target_bir_lowering=False, debug=False)

    def din(name, shape, dt=BF16):
        return nc.dram_tensor(name, shape, dt, kind="ExternalInput").ap()

    pnr_d = din("pnr", [NBLK, 128, PAIR, KC, NR])
    nrf_d = din("nrf", [NBLK, 128, PAIR, 3, D])
    ht_d = din("ht", [128, KC, BS])
    wnt_d = din("wnt", [128, KC, D])
    wrt_d = din("wrt", [128, KC, D])
    bnt_d = din("bnt", [128, KC], F32)
    brt_d = din("brt", [128, KC], F32)
    w12m_d = din("w12m", [128, G, KC, G2])
    msk_d = din("msk", [GROUPS, G2, NR])
    wng_d = din("wng", [128, KC2, 2, D])
    wrg_d = din("wrg", [128, KC2, 2, D])
    bng_d = din("bng", [1, 2, D])
    brg_d = din("brg", [1, 2, D])
    id_d = din("ident", [128, 128])
    ones_d = din("ones_row", [1, 128])

    nres_d = nc.dram_tensor("node_res", [BS, D], F32, kind="ExternalOutput").ap()
    rres_d = nc.dram_tensor("rela_res", [BS, D], F32, kind="ExternalOutput").ap()

    dma = nc.sync.dma_start
    dma_s = nc.gpsimd.dma_start

    with tile.TileContext(nc) as tc:
        with tc.tile_pool(name="const", bufs=1) as cp:
            # ---- early-critical loads on the sync queue ----
            ht_sb = cp.tile([128, KC, BS], BF16, tag="ht")
            dma(out=ht_sb, in_=ht_d)
            wnt_sb = cp.tile([128, KC, D], BF16, tag="wnt")
            wrt_sb = cp.tile([128, KC, D], BF16, tag="wrt")
            for kc in range(KC):
                dma(out=wnt_sb[:, kc, :], in_=wnt_d[:, kc, :])
                dma(out=wrt_sb[:, kc, :], in_=wrt_d[:, kc, :])
            bnt_sb = cp.tile([128, KC], F32, tag="bnt")
            dma(out=bnt_sb, in_=bnt_d)
            brt_sb = cp.tile([128, KC], F32, tag="brt")
            dma(out=brt_sb, in_=brt_d)
            ident = cp.tile([128, 128], BF16)
            dma(out=ident, in_=id_d)
            ones_row = cp.tile([1, 128], BF16)
            dma(out=ones_row, in_=ones_d)
            # ---- not needed immediately: idle gpsimd queue ----
            w12m_sb = cp.tile([128, G, KC, G2], BF16)
            dma_s(out=w12m_sb, in_=w12m_d)
            wng_sb = cp.tile([128, KC2, 2, D], BF16)
            wrg_sb = cp.tile([128, KC2, 2, D], BF16)
            bng_sb = cp.tile([1, 2, D], BF16)
            brg_sb = cp.tile([1, 2, D], BF16)

            # persistent result tiles
            # X rows per group live on partitions 0..G-1, group on free dim
            X_n_sb = cp.tile([G, GROUPS, D], BF16, tag="xnsb")
            X_r_sb = cp.tile([G, GROUPS, D], BF16, tag="xrsb")
            # transposed X columns for the GLU head: chunks 0..3 = Xn, 4..7 = Xr
            catXT = cp.tile([128, KC2, GROUPS, G], BF16, tag="catxt")
            # duplicated-pair transposed projections (bias source for the adds)
            nhT2 = cp.tile([128, KC, BS, 2], BF16, tag="nht2")
            rhT2 = cp.tile([128, KC, BS, 2], BF16, tag="rht2")

            # ---- prologue: nhT2 = (h @ W_h2node + b).T duplicated, same rela
            with tc.tile_pool(name="prolps", bufs=2, space="PSUM") as pp:
                for w_sb, b_sb, dst in ((wnt_sb, bnt_sb, nhT2),
                                        (wrt_sb, brt_sb, rhT2)):
                    for dc in range(KC):
                        # rhs = [ht | ht] via a step-0 outer dim: the matmul
                        # emits the dup pair directly
                        ps = pp.tile([128, 2, BS], F32, tag="pnh")
                        for kc in range(KC):
                            rhs2 = _view(ht_sb, kc * BS, [[0, 2], [1, BS]])
                            nc.tensor.matmul(
                                ps, w_sb[:, kc, dc * 128:(dc + 1) * 128],
                                rhs2,
                                start=(kc == 0), stop=(kc == KC - 1))
                        # one bias-folded copy, reordering (k, b) -> (b, k)
                        outap = _view(dst, dc * BS * 2, [[1, 2], [2, BS]])
                        nc.scalar.add(outap, ps, b_sb[:, dc:dc + 1])

            # ---- main loop: phase A(g) and phase C(g-1) interleaved;
            # GLU halves injected once their batch groups complete ----
            with (
                tc.tile_pool(name="pnrp", bufs=4) as pnrp,
                tc.tile_pool(name="nrfp", bufs=4) as nrfp,
                tc.tile_pool(name="args", bufs=3) as argp,
                tc.tile_pool(name="smax", bufs=2) as smp,
                tc.tile_pool(name="wcd", bufs=3) as wcdp,
                tc.tile_pool(name="masks", bufs=2) as mkp,
                tc.tile_pool(name="scps", bufs=2, space="PSUM") as scp,
                tc.tile_pool(name="xnps", bufs=2, space="PSUM") as xnp,
                tc.tile_pool(name="xrps", bufs=2, space="PSUM") as xrp,
                tc.tile_pool(name="ptrans", bufs=2, space="PSUM") as ptp,
            ):
                pools = dict(pnrp=pnrp, nrfp=nrfp,
                             argp=argp, smp=smp, wcdp=wcdp, mkp=mkp,
                             scp=scp, xnp=xnp, xrp=xrp, ptp=ptp)
                cs = dict(ident=ident, ones_row=ones_row, w12m_sb=w12m_sb,
                          nhT2=nhT2, rhT2=rhT2,
                          X_n_sb=X_n_sb, X_r_sb=X_r_sb, catXT=catXT,
                          pnr_d=pnr_d, nrf_d=nrf_d, msk_d=msk_d,
                          wng_sb=wng_sb, wrg_sb=wrg_sb,
                          bng_sb=bng_sb, brg_sb=brg_sb,
                          nres_d=nres_d, rres_d=rres_d)
                wprev = None
                xprev = None
                for g in range(GROUPS):
                    # GLU weights: deferred and spread so no single group's
                    # streams contend with them; gpsimd queue is otherwise idle
                    if g == 1:
                        dma_s(out=bng_sb, in_=bng_d)
                        dma_s(out=brg_sb, in_=brg_d)
                    elif g == 2:
                        dma_s(out=wng_sb, in_=wng_d)
                    elif g == 3:
                        dma_s(out=wrg_sb, in_=wrg_d)
                    sc = None
                    for j in range(G // PAIR):
                        if wprev is not None:
                            xprev = _phase_c_block(nc, dma, g - 1, j, pools,
                                                   cs, wprev, xprev)
                        sc = _phase_a_block(nc, dma, g, j, pools, cs, sc)
                    if wprev is not None:
                        _phase_c_post(nc, g - 1, pools, cs, xprev)
                        xprev = None
                    wprev = _phase_b(nc, dma_s, g, pools, cs, sc)
                g = GROUPS
                for j in range(G // PAIR):
                    xprev = _phase_c_block(nc, dma, g - 1, j, pools, cs,
                                           wprev, xprev)
                _phase_c_post(nc, g - 1, pools, cs, xprev)

            # ---- GLU head (gate-b first so sigmoid overlaps gate-a MMs) ----
            with (
                tc.tile_pool(name="esb", bufs=1) as ep,
                tc.tile_pool(name="etp", bufs=2, space="PSUM") as ept,
                tc.tile_pool(name="ebp", bufs=2, space="PSUM") as epb,
            ):
                ng_ps = epb.tile([BS, 2, D], F32, tag="ebig")
                for hh in (1, 0):
                    for c in range(KC2):
                        nc.tensor.matmul(ng_ps[:, hh, :], catXT[:, c, :, :],
                                         wng_sb[:, c, hh, :],
                                         start=(c == 0), stop=False)
                    nc.tensor.matmul(ng_ps[:, hh, :], ones_row[:, :BS],
                                     bng_sb[:, hh, :], start=False, stop=True)
                    if hh == 1:
                        sigN = ep.tile([BS, D], F32, tag="sigN")
                        nc.scalar.activation(sigN, ng_ps[:, 1, :], AF.Sigmoid)
                # rela gate: Xr-chunk MMs issue before the nres dependency
                rg_ps = epb.tile([BS, 2, D], F32, tag="ebig")
                for hh in (1, 0):
                    for c in range(KC):
                        nc.tensor.matmul(rg_ps[:, hh, :],
                                         catXT[:, KC + c, :, :],
                                         wrg_sb[:, c, hh, :],
                                         start=(c == 0), stop=False)
                nres_bf = ep.tile([BS, D], BF16, tag="nresbf")
                nc.vector.tensor_mul(nres_bf, ng_ps[:, 0, :], sigN)
                nres_sb = ep.tile([BS, D], F32, tag="nres")
                nc.vector.tensor_mul(nres_sb, ng_ps[:, 0, :], sigN)
                dma(out=nres_d, in_=nres_sb)
                NT = ep.tile([128, KC, BS], BF16, tag="nt")
                for c in range(KC):
                    tp = ept.tile([128, BS], BF16, tag="et")
                    nc.tensor.transpose(tp, nres_bf[:, c * 128:(c + 1) * 128],
                                        ident[:BS, :BS])
                    nc.scalar.copy(NT[:, c, :], tp)
                for hh in (1, 0):
                    for c in range(KC):
                        nc.tensor.matmul(rg_ps[:, hh, :], NT[:, c, :],
                                         wrg_sb[:, KC + c, hh, :],
                                         start=False, stop=False)
                    nc.tensor.matmul(rg_ps[:, hh, :], ones_row[:, :BS],
                                     brg_sb[:, hh, :], start=False, stop=True)
                    if hh == 1:
                        sigR = ep.tile([BS, D], F32, tag="sigR")
                        nc.scalar.activation(sigR, rg_ps[:, 1, :], AF.Sigmoid)
                rres_sb = ep.tile([BS, D], F32, tag="rres")
                nc.vector.tensor_mul(rres_sb, rg_ps[:, 0, :], sigR)
                dma(out=rres_d, in_=rres_sb)

    nc.compile()
    return nc


def _phase_a_block(nc, dma, g, j, pools, cs, sc):
    """One PAIR block of phase A: stream pnf/prf (d-major bf16), add
    projections, one fused tanh, fused score matmuls into [G2, NR] PSUM."""
    pnrp = pools["pnrp"]; argp = pools["argp"]
    scp = pools["scp"]
    w12m_sb = cs["w12m_sb"]
    nhT2 = cs["nhT2"]; rhT2 = cs["rhT2"]
    pnr_d = cs["pnr_d"]

    if sc is None:
        sc = scp.tile([G2, NR], F32, tag="sc")
    b0 = g * G + j * PAIR
    blk = b0 // PAIR
    pnr2 = pnrp.tile([128, PAIR, KC, NR], BF16, tag="pnr2")
    dma(out=pnr2, in_=pnr_d[blk])
    argNR = argp.tile([128, PAIR, KC, NR], BF16, tag="argNR")
    # adds: [128, PAIR, X/2, 2]-viewed, node_h broadcast along free dim
    # (dup-pair innermost keeps the packed-2byte fast path on DVE)
    for c in range(KC):
        outN = _view(argNR, c * NR, [[KC * NR, PAIR], [2, N // 2], [1, 2]])
        inN = _view(pnr2, c * NR, [[KC * NR, PAIR], [2, N // 2], [1, 2]])
        bcN = _view(nhT2, c * BS * 2 + b0 * 2,
                    [[2, PAIR], [0, N // 2], [1, 2]])
        nc.vector.tensor_add(outN, inN, bcN)
        outR = _view(argNR, c * NR + N,
                     [[KC * NR, PAIR], [2, R // 2], [1, 2]])
        inR = _view(pnr2, c * NR + N,
                    [[KC * NR, PAIR], [2, R // 2], [1, 2]])
        bcR = _view(rhT2, c * BS * 2 + b0 * 2,
                    [[2, PAIR], [0, R // 2], [1, 2]])
        nc.vector.tensor_add(outR, inR, bcR)
    if g == 0:
        # pipeline-fill: split tanh so the first score MMs start sooner
        for half in range(2):
            hv = _view(argNR, half * 2 * KC * NR,
                       [[KC * NR, 2], [NR, KC], [1, NR]])
            nc.scalar.activation(hv, hv, AF.Tanh)
    else:
        nc.scalar.activation(argNR, argNR, AF.Tanh)
    for i in range(PAIR):
        jj = j * PAIR + i
        for c in range(KC):
            mm = jj * KC + c
            nc.tensor.matmul(sc, w12m_sb[:, jj, c, :], argNR[:, i, c, :],
                             start=(mm == 0), stop=(mm == G * KC - 1))
    return sc


def _phase_b(nc, dma_s, g, pools, cs, sc):
    """One fused masked softmax over [G2, NR] (junk regions zeroed by the
    host-fused mask); weights transposed back and diagonalized into
    [128, G, G] tiles for phase C."""
    smp = pools["smp"]; wcdp = pools["wcdp"]; mkp = pools["mkp"]
    ptp = pools["ptp"]
    ident = cs["ident"]; msk_d = cs["msk_d"]

    m_t = mkp.tile([G2, NR], BF16, tag="mt")
    dma_s(out=m_t, in_=msk_d[g])

    mneg = smp.tile([G2, 1], F32, tag="mneg")
    nc.vector.tensor_reduce(out=mneg, in_=sc, axis=AX.X, op=ALU.max,
                            negate=True)
    E = smp.tile([G2, NR], BF16, tag="E")
    nc.scalar.activation(E, sc, AF.Exp, bias=mneg)
    EM = smp.tile([G2, NR], BF16, tag="EM")
    nc.vector.tensor_mul(EM, E, m_t)
    S = smp.tile([G2, 1], F32, tag="S")
    nc.vector.reduce_sum(out=S, in_=EM, axis=AX.X)
    rS = smp.tile([G2, 1], F32, tag="rS")
    nc.vector.reciprocal(rS, S)
    W_w = smp.tile([G2, NR], BF16, tag="W")
    nc.vector.tensor_scalar_mul(W_w, EM, rS)

    out_cols = []
    for s, csel in ((0, 0), (1, G), (2, G)):  # node, rela0, rela1
        wT = ptp.tile([128, G2], BF16, tag="wT")
        nc.tensor.transpose(wT, W_w[:, s * 128:(s + 1) * 128],
                            ident[:G2, :G2])
        WCd = wcdp.tile([128, G, G], BF16, tag=f"wcd{s}")
        nc.vector.memset(WCd, 0.0)
        diag = _view(WCd, 0, [[G + 1, G]])
        nc.vector.tensor_copy(diag, wT[:, csel:csel + G])
        out_cols.append(WCd)
    return out_cols  # [WnCd, Wr0Cd, Wr1Cd]


def _phase_c_block(nc, dma, g, j, pools, cs, wcols, xps):
    """One PAIR block of phase C: diagonalized weights accumulate all G
    batch rows of the group into [G, D] PSUM tiles."""
    WnCd, Wr0Cd, Wr1Cd = wcols
    nrfp = pools["nrfp"]
    xnp = pools["xnp"]; xrp = pools["xrp"]

    if xps is None:
        Xn_ps = xnp.tile([G, D], F32, tag="xn")
        Xr_ps = xrp.tile([G, D], F32, tag="xr")
    else:
        Xn_ps, Xr_ps = xps
    b0 = g * G + j * PAIR
    blk = b0 // PAIR
    nrf2 = nrfp.tile([128, PAIR, 3, D], BF16, tag="nrf2")
    dma(out=nrf2, in_=cs["nrf_d"][blk])
    for i in range(PAIR):
        jj = j * PAIR + i
        nc.tensor.matmul(Xn_ps, WnCd[:, jj, :], nrf2[:, i, 0, :],
                         start=(jj == 0), stop=(jj == G - 1))
        nc.tensor.matmul(Xr_ps, Wr0Cd[:, jj, :], nrf2[:, i, 1, :],
                         start=(jj == 0), stop=False)
        nc.tensor.matmul(Xr_ps, Wr1Cd[:, jj, :], nrf2[:, i, 2, :],
                         start=False, stop=(jj == G - 1))
    return Xn_ps, Xr_ps


def _phase_c_post(nc, g, pools, cs, xps):
    """Land the group's X rows in SBUF and build catXT columns."""
    Xn_ps, Xr_ps = xps
    ptp = pools["ptp"]
    X_n_sb = cs["X_n_sb"]; X_r_sb = cs["X_r_sb"]; catXT = cs["catXT"]
    ident = cs["ident"]
    nc.vector.tensor_copy(X_n_sb[:, g, :], Xn_ps)
    nc.vector.tensor_copy(X_r_sb[:, g, :], Xr_ps)
    for c in range(KC):
        tpn = ptp.tile([128, G2], BF16, tag="wT")
        nc.tensor.transpose(tpn[:, 0:G], X_n_sb[:, g, c * 128:(c + 1) * 128],
                            ident[:G, :G])
        nc.scalar.copy(catXT[:, c, g, :], tpn[:, 0:G])
        tpr = ptp.tile([128, G2], BF16, tag="wT")
        nc.tensor.transpose(tpr[:, 0:G], X_r_sb[:, g, c * 128:(c + 1) * 128],
                            ident[:G, :G])
        nc.scalar.copy(catXT[:, KC + c, g, :], tpr[:, 0:G])



def make_in_maps(inputs):
    """Shard full inputs into 8 per-core input dicts (host-side layout and
    dtype prep only; all math runs on device)."""
    f32 = np.float32

    def bf(x):
        return np.ascontiguousarray(np.asarray(x, dtype=f32).astype(NPBF))

    h = np.asarray(inputs["h"], dtype=f32)
    pnf = np.asarray(inputs["p_node_feats"], dtype=f32)
    nf = np.asarray(inputs["node_feats"], dtype=f32)
    prf = np.asarray(inputs["p_rela_feats"], dtype=f32)
    rf = np.asarray(inputs["rela_feats"], dtype=f32)
    am = np.asarray(inputs["att_masks"], dtype=f32)
    rm = np.asarray(inputs["rela_masks"], dtype=f32)

    def shuf_p(x, L, dt=NPBF):  # [BS, L, D] -> [NBLK, 128, PAIR, KC, L]
        x = x.reshape(NBLK, PAIR, L, KC, 128)
        return np.ascontiguousarray(x.transpose(0, 4, 1, 3, 2).astype(dt))

    def shuf_n(x):  # [BS, N, D] -> [NBLK, 128, PAIR, D]
        x = x.reshape(NBLK, PAIR, N, D)
        return bf(x.transpose(0, 2, 1, 3))

    def shuf_r(x):  # [BS, R, D] -> [NBLK, 128, PAIR, 2, D]
        x = x.reshape(NBLK, PAIR, 2, 128, D)
        return bf(x.transpose(0, 3, 1, 2, 4))

    # fused one-hot score weights: cols 0:G node (w_alpha1), G:2G rela
    w1r = np.asarray(inputs["w_alpha1"], dtype=f32).reshape(KC, 128)
    w2r = np.asarray(inputs["w_alpha2"], dtype=f32).reshape(KC, 128)
    w12m = np.zeros((128, G, KC, G2), dtype=f32)
    for jj in range(G):
        w12m[:, jj, :, jj] = w1r.T
        w12m[:, jj, :, G + jj] = w2r.T

    wng = np.asarray(inputs["W_ng"], dtype=f32)
    wrg = np.asarray(inputs["W_rg"], dtype=f32)
    shared = {
        "wnt": bf(np.asarray(inputs["W_h2node"], dtype=f32)
                  .reshape(KC, 128, D).transpose(1, 0, 2)),
        "wrt": bf(np.asarray(inputs["W_h2rela"], dtype=f32)
                  .reshape(KC, 128, D).transpose(1, 0, 2)),
        "bnt": np.ascontiguousarray(np.asarray(inputs["b_h2node"], dtype=f32)
                                    .reshape(KC, 128).transpose(1, 0)),
        "brt": np.ascontiguousarray(np.asarray(inputs["b_h2rela"], dtype=f32)
                                    .reshape(KC, 128).transpose(1, 0)),
        "w12m": bf(w12m),
        "wng": bf(wng.reshape(KC2, 128, 2, D).transpose(1, 0, 2, 3)),
        "wrg": bf(wrg.reshape(KC2, 128, 2, D).transpose(1, 0, 2, 3)),
        "bng": bf(np.asarray(inputs["b_ng"], dtype=f32).reshape(1, 2, D)),
        "brg": bf(np.asarray(inputs["b_rg"], dtype=f32).reshape(1, 2, D)),
        "ident": bf(np.eye(128, dtype=f32)),
        "ones_row": bf(np.ones((1, 128), dtype=f32)),
    }
    in_maps = []
    for c in range(NCORES):
        s = slice(c * BS, (c + 1) * BS)
        # fused zero-padded softmax mask per group
        msk = np.zeros((GROUPS, G2, NR), dtype=f32)
        msk[:, 0:G, 0:N] = am[s].reshape(GROUPS, G, N)
        msk[:, G:G2, N:NR] = rm[s].reshape(GROUPS, G, R)
        in_maps.append({
            "pnr": np.ascontiguousarray(np.concatenate(
                [shuf_p(pnf[s], N), shuf_p(prf[s], R)], axis=-1)),
            "nrf": np.ascontiguousarray(np.concatenate(
                [shuf_n(nf[s])[:, :, :, None, :], shuf_r(rf[s])], axis=3)),
            "ht": bf(h[s].reshape(BS, KC, 128).transpose(2, 1, 0)),
            "msk": bf(msk),
            **shared,
        })
    return in_maps


_NC_CACHE = None
LAST_RESULTS = None  # BassKernelResults of the most recent kernel() call


def kernel(**inputs):
    global _NC_CACHE, LAST_RESULTS
    if _NC_CACHE is None:
        _NC_CACHE = build_program()
    nc = _NC_CACHE
    in_maps = make_in_maps(inputs)
    import os
    trace = os.environ.get("BASS_KERNEL_TRACE", "0") == "1"
    res = run_bass_kernel_spmd(nc, in_maps, core_ids=list(range(NCORES)),
                               trace=trace)
    LAST_RESULTS = res
    node_res = np.concatenate([r["node_res"] for r in res.results], axis=0)
    rela_res = np.concatenate([r["rela_res"] for r in res.results], axis=0)
    return node_res, rela_res
